# revision 1
# baseline (speedup 1.0000x reference)
"""Trainium2 Bass kernel for nn_DecoderLayer_15891378995467.

Fast-weight (linear-attention) decoder layer:
  qkv = h @ W_qkv.T ; q,k1,k2,v per head ; phi = L1-normalized elu+1
  two causal linear attentions mixed by pi ; output proj ; residual ; LayerNorm.

Sharding: data-parallel over batch (64 = 8 cores x 8 local batches).
All matmuls run in bf16 (fp32 PSUM accumulation); residual + LayerNorm in fp32.

Per-core layouts (host prepares):
  hT   [8, 1024, 256] bf16 : h[:, core_b, :].T  per local batch (m-major)
  hN   [256, 8, 1024] f32  : natural slice for the residual
  w1   [1024, 4096]  bf16  : W_qkv reordered to [q|k1|k2|v] x head-major, transposed
  w2   [1024, 1024]  bf16  : W_o.T (rows n = head*128 + dv)
  pc   [256, 16]     f32   : SCALE*clip(pi0).T and SCALE*(1-clip(pi0)).T
  mask0 [128, 256] f32, mask1 [128, 128] f32 : causal masks for scoresT tiles
"""

import sys

if "/opt/trn_rl_repo" not in sys.path:
    sys.path.insert(0, "/opt/trn_rl_repo")

import numpy as np
import ml_dtypes

import concourse.bass as bass
import concourse.mybir as mybir
import concourse.tile as tile
from concourse.vector_clock import ScopedClock, VectorClock
from concourse.bass_utils import run_bass_kernel_spmd

F32 = mybir.dt.float32
BF16 = mybir.dt.bfloat16
AX = mybir.AxisListType
ALU = mybir.AluOpType
ACTF = mybir.ActivationFunctionType

H, DH, DM = 8, 128, 1024
SLEN, BSZ = 256, 64
NCORES = 8
BLOC = BSZ // NCORES  # 8 local batches per core
SCALE = 1.0 / DH**0.5
LN_EPS = 1e-5
NQKV = 4 * DM  # 4096


class SplitDrainTileContext(tile.TileContext):
    """This walrus build only encodes one sem-wait per Drain; split the
    tail drain into a chain of single-wait drains."""

    def _drain_and_barrier(self, tick_clock, wait_clock):
        vc_full = tick_clock.global_clock
        n = len(vc_full)
        procs = [i for i in range(n) if vc_full[i] > 0]
        groups = [procs[i : i + 1] for i in range(len(procs))] or [[]]
        for grp in groups:
            part = VectorClock([0] * n)
            for p in grp:
                part.require_at_least(p, vc_full[p])
            d = self.nc.sync.drain()
            wait_clock.add_sem_waits(d.ins, ScopedClock({None: part}))
        self.nc.all_engine_barrier()
        assert self.sems is not None
        popped = self.nc._tile_sem_poison_stack.pop()
        assert popped is self._sem_poison
        self.nc.clear_and_free_semaphores(list(self.sems.allocated().values()))
        self.nc.all_engine_barrier()
        self._split_multiwaits()

    def _split_multiwaits(self):
        """Walrus here encodes at most one sem-wait per instruction; hoist
        extra waits onto same-engine NOPs inserted just before."""
        fn = self.nc.m.functions[0]
        seq = 0
        for bb in fn.blocks:
            insts = list(bb.instructions)
            if not any(
                i.sync_info is not None and len(i.sync_info.on_wait) > 1
                for i in insts
            ):
                continue
            new_insts = []
            for inst in insts:
                si = inst.sync_info
                if si is not None and len(si.on_wait) > 1:
                    waits = list(si.on_wait)
                    eng = self.nc.engines[inst.engine]
                    for w in waits[:-1]:
                        seq += 1
                        bi = eng.nop()
                        nop = bi.ins
                        cur = self.nc.cur_bb.bb.instructions
                        assert cur and cur[-1] is nop
                        cur.pop()
                        nop.sync_info = mybir.SyncInfo(on_wait=[w], on_update=[])
                        new_insts.append(nop)
                    inst.sync_info = mybir.SyncInfo(
                        on_wait=[waits[-1]], on_update=list(si.on_update)
                    )
                new_insts.append(inst)
            try:
                bb.instructions[:] = new_insts
            except TypeError:
                bb.instructions = new_insts


def build_program(passes=1):
    nc = bass.Bass("TRN2", target_bir_lowering=False, debug=False, num_devices=NCORES)

    hT = nc.declare_dram_parameter("hT", [BLOC, DM, SLEN], BF16, isOutput=False)
    hN = nc.declare_dram_parameter("hN", [SLEN, BLOC, DM], F32, isOutput=False)
    w1 = nc.declare_dram_parameter("w1", [DM, NQKV], BF16, isOutput=False)
    w2 = nc.declare_dram_parameter("w2", [DM, DM], BF16, isOutput=False)
    pc = nc.declare_dram_parameter("pc", [SLEN, 2 * H], F32, isOutput=False)
    mask0 = nc.declare_dram_parameter("mask0", [128, 256], F32, isOutput=False)
    mask1 = nc.declare_dram_parameter("mask1", [128, 128], F32, isOutput=False)
    ident = nc.declare_dram_parameter("ident", [128, 128], F32, isOutput=False)
    identb = nc.declare_dram_parameter("identb", [128, 128], BF16, isOutput=False)
    sel = nc.declare_dram_parameter("sel", [128, 24, 24], BF16, isOutput=False)
    out = nc.declare_dram_parameter("out", [SLEN, BLOC, DM], F32, isOutput=True)

    with SplitDrainTileContext(nc) as tc:
        for _ in range(passes):
            _emit(nc, tc, hT, hN, w1, w2, pc, mask0, mask1, ident, identb, sel, out)
    return nc


def _emit(nc, tc, hT, hN, w1, w2, pc, mask0, mask1, ident, identb, sel, out):
    from contextlib import ExitStack

    ctx = ExitStack()
    with ctx:
        singles = ctx.enter_context(tc.tile_pool(name="singles", bufs=1))
        hT_pool = ctx.enter_context(tc.tile_pool(name="hT", bufs=2))
        qk_pool = ctx.enter_context(tc.tile_pool(name="qk", bufs=2))
        v_pool = ctx.enter_context(tc.tile_pool(name="v", bufs=2))
        t_pool = ctx.enter_context(tc.tile_pool(name="tscratch", bufs=2))
        sc_pool = ctx.enter_context(tc.tile_pool(name="scores", bufs=4))
        lon_pool = ctx.enter_context(tc.tile_pool(name="lon", bufs=8))
        dg_pool = ctx.enter_context(tc.tile_pool(name="diag", bufs=4))
        ly_pool = ctx.enter_context(tc.tile_pool(name="ly", bufs=2))
        x_pool = ctx.enter_context(tc.tile_pool(name="x", bufs=3))
        st_pool = ctx.enter_context(tc.tile_pool(name="stats", bufs=4))
        ps_qkv = ctx.enter_context(tc.tile_pool(name="ps_qkv", bufs=2, space="PSUM"))
        ps_sc = ctx.enter_context(tc.tile_pool(name="ps_sc", bufs=2, space="PSUM"))
        ps_lo = ctx.enter_context(tc.tile_pool(name="ps_lo", bufs=3, space="PSUM"))
        ps_op = ctx.enter_context(tc.tile_pool(name="ps_op", bufs=1, space="PSUM"))

        # --- persistent weights / constants ---
        w1_s = singles.tile([128, 8, NQKV], BF16)
        w1_v = w1.rearrange("(c p) n -> c p n", p=128)
        for c in range(8):
            nc.sync.dma_start(out=w1_s[:, c, :], in_=w1_v[c])
        w2_s = singles.tile([128, 8, DM], BF16)
        w2_v = w2.rearrange("(c p) n -> c p n", p=128)
        for c in range(8):
            nc.sync.dma_start(out=w2_s[:, c, :], in_=w2_v[c])
        m1_s = singles.tile([128, 128], F32)
        nc.sync.dma_start(out=m1_s[:], in_=mask1[:])
        id_s = singles.tile([128, 128], F32)
        nc.sync.dma_start(out=id_s[:], in_=ident[:])
        idb_s = singles.tile([128, 128], BF16)
        nc.sync.dma_start(out=idb_s[:], in_=identb[:])
        pc_s = singles.tile([128, 2, 2 * H], F32)
        pc_v = pc.rearrange("(t p) n -> t p n", p=128)
        for t in range(2):
            nc.sync.dma_start(out=pc_s[:, t, :], in_=pc_v[t])
        eps_s = singles.tile([128, 1], F32)
        nc.vector.memset(eps_s[:], LN_EPS)
        sel_s = singles.tile([128, 24, 24], BF16)
        nc.sync.dma_start(out=sel_s[:], in_=sel[:])

        state = [None] * BLOC  # per-batch dict of tiles for the back stage

        def stage_front(b):
            """QKV projection (q/k transposed, v natural) + phi + stats."""
            hT_t = hT_pool.tile([128, 8, SLEN], BF16, tag="hT")
            hT_v = hT[b].rearrange("(c p) l -> p c l", p=128)
            nc.sync.dma_start(out=hT_t[:], in_=hT_v)

            qk_t = qk_pool.tile([128, 24, SLEN], BF16, tag="qk")
            v_t = v_pool.tile([128, 2, DM], BF16, tag="v")
            hN_t = []
            for lt in range(2):
                x_t = x_pool.tile([128, DM], F32, tag="x")
                nc.sync.dma_start(
                    out=x_t[:], in_=hN[lt * 128 : (lt + 1) * 128, b, :]
                )
                hN_t.append(x_t)

            # q,k1,k2 in transposed orientation: out[n_blk(128), l(256)]
            for j in range(24):
                ps = ps_qkv.tile([128, 512], F32, tag="ps_qkv")
                for mc in range(8):
                    nc.tensor.matmul(
                        ps[:, 0:SLEN],
                        lhsT=w1_s[:, mc, j * 128 : (j + 1) * 128],
                        rhs=hT_t[:, mc, :],
                        start=(mc == 0),
                        stop=(mc == 7),
                    )
                if j % 2 == 0:
                    nc.vector.tensor_copy(qk_t[:, j, :], ps[:, 0:SLEN])
                else:
                    nc.scalar.copy(qk_t[:, j, :], ps[:, 0:SLEN])
            # v natural: out[l_tile(128), n(512)]
            for lt in range(2):
                for vt in range(2):
                    ps = ps_qkv.tile([128, 512], F32, tag="ps_qkv")
                    for mc in range(8):
                        nc.tensor.matmul(
                            ps[:],
                            lhsT=hT_t[:, mc, lt * 128 : (lt + 1) * 128],
                            rhs=w1_s[:, mc, 3 * DM + vt * 512 : 3 * DM + (vt + 1) * 512],
                            start=(mc == 0),
                            stop=(mc == 7),
                        )
                    if vt == 0:
                        nc.vector.tensor_copy(v_t[:, lt, 0:512], ps[:])
                    else:
                        nc.scalar.copy(v_t[:, lt, 512:1024], ps[:])

            # phi: u = exp(min(x,0)) + relu(x) over the whole transposed block
            reg = qk_t[:, :, :]
            tt = t_pool.tile([128, 24, SLEN], BF16, tag="t")
            nc.vector.tensor_scalar_min(tt[:], reg, 0.0)
            nc.scalar.activation(tt[:], tt[:], ACTF.Exp)
            nc.vector.scalar_tensor_tensor(
                reg, reg, 0.0, tt[:], op0=ALU.max, op1=ALU.add
            )

            # per-(l, block) sums: accumulate 24 one-hot-selector matmuls
            # into a single [24, 256] PSUM tile (row j = sums of block j)
            ps_sum = ps_sc.tile([128, 256], F32, tag="ps_sc")
            for j in range(24):
                nc.tensor.matmul(
                    ps_sum[0:24, :],
                    lhsT=sel_s[:, j, :],
                    rhs=qk_t[:, j, :],
                    start=(j == 0),
                    stop=(j == 23),
                )
            s_t = st_pool.tile([24, 256], F32, tag="sums")
            nc.vector.tensor_copy(s_t[:], ps_sum[0:24, :])
            # transpose sums to [l(128), 24] per l_tile, then reciprocal
            rT_t = st_pool.tile([128, 2, 24], F32, tag="recipT")
            for lt in range(2):
                ps_t = ps_sc.tile([128, 256], F32, tag="ps_sc")
                nc.tensor.transpose(
                    ps_t[:, 0:24], s_t[:, lt * 128 : (lt + 1) * 128], id_s[0:24, 0:24]
                )
                nc.vector.reciprocal(rT_t[:, lt, :], ps_t[:, 0:24])
            # c1/c2 mixing coefficients [l(128), H] per l_tile
            c12 = []
            for lt in range(2):
                c1 = st_pool.tile([128, H], F32, tag="c1")
                nc.vector.tensor_tensor(
                    c1[:], pc_s[:, lt, 0:H], rT_t[:, lt, 0:H], op=ALU.mult
                )
                c2 = st_pool.tile([128, H], F32, tag="c2")
                nc.vector.tensor_tensor(
                    c2[:], pc_s[:, lt, H : 2 * H], rT_t[:, lt, 0:H], op=ALU.mult
                )
                c12.append((c1, c2))

            state[b] = dict(v=v_t, qk=qk_t, rT=rT_t, c12=c12, hN=hN_t)

        def stage_back(b):
            """Attention + pi-mix (diag matmul) + O-proj + residual + LN."""
            st = state[b]
            v_t = st["v"]
            qk_t = st["qk"]
            rT_t = st["rT"]
            c12 = st["c12"]
            ly_t = ly_pool.tile([128, H, SLEN], BF16, tag="ly")
            for h in range(H):
                lo_nat = []  # [i][lt] sbuf tiles [l_tile, dv]
                for i in range(2):
                    jk = 8 + i * 8 + h
                    rcol = 8 + i * 8 + h
                    ps0 = ps_sc.tile([128, 256], F32, tag="ps_sc")
                    nc.tensor.matmul(
                        ps0[:],
                        lhsT=qk_t[:, jk, 0:128],
                        rhs=qk_t[:, h, :],
                        start=True,
                        stop=True,
                    )
                    ps1 = ps_sc.tile([128, 256], F32, tag="ps_sc")
                    nc.tensor.matmul(
                        ps1[:, 0:128],
                        lhsT=qk_t[:, jk, 128:256],
                        rhs=qk_t[:, h, 128:256],
                        start=True,
                        stop=True,
                    )
                    # masked / scaled copies of scoresT
                    sc00 = sc_pool.tile([128, 128], BF16, tag="sc00")
                    nc.vector.scalar_tensor_tensor(
                        sc00[:], ps0[:, 0:128], rT_t[:, 0, rcol : rcol + 1],
                        m1_s[:], op0=ALU.mult, op1=ALU.mult,
                    )
                    sc01 = sc_pool.tile([128, 128], BF16, tag="sc01")
                    nc.scalar.activation(
                        sc01[:], ps0[:, 128:256], ACTF.Copy,
                        scale=rT_t[:, 0, rcol : rcol + 1],
                    )
                    sc11 = sc_pool.tile([128, 128], BF16, tag="sc11")
                    nc.vector.scalar_tensor_tensor(
                        sc11[:], ps1[:, 0:128], rT_t[:, 1, rcol : rcol + 1],
                        m1_s[:], op0=ALU.mult, op1=ALU.mult,
                    )
                    # apply (natural orientation): lo_i[l_tile, dv]
                    blk = slice(h * 128, (h + 1) * 128)
                    lp0 = ps_lo.tile([128, 128], F32, tag="ps_lo")
                    nc.tensor.matmul(
                        lp0[:], lhsT=sc00[:], rhs=v_t[:, 0, blk],
                        start=True, stop=True,
                    )
                    lp1 = ps_lo.tile([128, 128], F32, tag="ps_lo")
                    nc.tensor.matmul(
                        lp1[:], lhsT=sc01[:], rhs=v_t[:, 0, blk],
                        start=True, stop=False,
                    )
                    nc.tensor.matmul(
                        lp1[:], lhsT=sc11[:], rhs=v_t[:, 1, blk],
                        start=False, stop=True,
                    )
                    l0 = lon_pool.tile([128, 128], BF16, tag="lon")
                    l1 = lon_pool.tile([128, 128], BF16, tag="lon")
                    if i == 0:
                        nc.vector.tensor_copy(l0[:], lp0[:])
                        nc.scalar.copy(l1[:], lp1[:])
                    else:
                        nc.scalar.copy(l0[:], lp0[:])
                        nc.vector.tensor_copy(l1[:], lp1[:])
                    lo_nat.append((l0, l1))
                # pi-mix via diagonal matmuls: layer_outT[dv, l] accumulated
                for lt in range(2):
                    mps = ps_lo.tile([128, 128], F32, tag="ps_lo")
                    for i in range(2):
                        ci = c12[lt][i]
                        dg = dg_pool.tile([128, 128], BF16, tag="dg")
                        nc.vector.tensor_scalar_mul(
                            dg[:], idb_s[:], ci[:, h : h + 1]
                        )
                        nc.tensor.matmul(
                            mps[:],
                            lhsT=lo_nat[i][lt][:],
                            rhs=dg[:],
                            start=(i == 0),
                            stop=(i == 1),
                        )
                    if lt == 0:
                        nc.vector.tensor_copy(
                            ly_t[:, h, lt * 128 : (lt + 1) * 128], mps[:]
                        )
                    else:
                        nc.scalar.copy(
                            ly_t[:, h, lt * 128 : (lt + 1) * 128], mps[:]
                        )

            # output projection + residual + LN per l_tile
            for lt in range(2):
                x_t = st["hN"][lt]
                acc = st_pool.tile([128, 4], F32, tag="acc")
                for mo in range(2):
                    ps = ps_op.tile([128, 512], F32, tag="ps_op")
                    for h in range(H):
                        nc.tensor.matmul(
                            ps[:],
                            lhsT=ly_t[:, h, lt * 128 : (lt + 1) * 128],
                            rhs=w2_s[:, h, mo * 512 : (mo + 1) * 512],
                            start=(h == 0),
                            stop=(h == 7),
                        )
                    nc.vector.scalar_tensor_tensor(
                        x_t[:, mo * 512 : (mo + 1) * 512],
                        ps[:],
                        0.0,
                        x_t[:, mo * 512 : (mo + 1) * 512],
                        op0=ALU.add,
                        op1=ALU.add,
                        accum_out=acc[:, mo : mo + 1],
                    )
                sq = t_pool.tile([128, DM], F32, tag="t")
                ssq = st_pool.tile([128, 1], F32, tag="ssq")
                nc.scalar.activation(
                    sq[:], x_t[:], ACTF.Square, accum_out=ssq[:]
                )
                mu = st_pool.tile([128, 1], F32, tag="mu")
                nc.vector.tensor_scalar(
                    mu[:], acc[:, 0:1], 1.0 / DM, None, op0=ALU.mult
                )
                nc.vector.scalar_tensor_tensor(
                    mu[:], acc[:, 1:2], 1.0 / DM, mu[:],
                    op0=ALU.mult, op1=ALU.add,
                )
                mu2 = st_pool.tile([128, 1], F32, tag="mu2")
                nc.vector.tensor_tensor(mu2[:], mu[:], mu[:], op=ALU.mult)
                var = st_pool.tile([128, 1], F32, tag="var")
                nc.vector.scalar_tensor_tensor(
                    var[:], ssq[:], 1.0 / DM, mu2[:],
                    op0=ALU.mult, op1=ALU.subtract,
                )
                sd = st_pool.tile([128, 1], F32, tag="sd")
                nc.scalar.activation(sd[:], var[:], ACTF.Sqrt, bias=eps_s[:])
                rstd = st_pool.tile([128, 1], F32, tag="rstd")
                nc.vector.reciprocal(rstd[:], sd[:])
                nc.vector.tensor_scalar(
                    x_t[:], x_t[:], mu[:], rstd[:],
                    op0=ALU.subtract, op1=ALU.mult,
                )
                nc.sync.dma_start(
                    out=out[lt * 128 : (lt + 1) * 128, b, :], in_=x_t[:]
                )
            state[b] = None

        # software pipeline: front(b) || back(b-1)
        for b in range(BLOC + 1):
            if b < BLOC:
                stage_front(b)
            if b >= 1:
                stage_back(b - 1)


_PROGRAM_CACHE = {}


def _get_program():
    if "nc" not in _PROGRAM_CACHE:
        _PROGRAM_CACHE["nc"] = build_program()
    return _PROGRAM_CACHE["nc"]


def prepare_inputs(h, W_qkv, W_o, pi0, ln_gamma, ln_beta):
    """Host-side shard + relayout. Returns per-core input maps."""
    h = np.ascontiguousarray(h, dtype=np.float32)
    W_qkv = np.asarray(W_qkv, dtype=np.float32)
    W_o = np.asarray(W_o, dtype=np.float32)
    pi0 = np.asarray(pi0, dtype=np.float32)

    # W1: [m, g*1024 + h*128 + d] <- W_qkv[h*512 + g*128 + d, m]
    w1 = np.ascontiguousarray(
        W_qkv.reshape(H, 4, DH, DM).transpose(3, 1, 0, 2).reshape(DM, NQKV)
    ).astype(ml_dtypes.bfloat16)
    w2 = np.ascontiguousarray(W_o.T).astype(ml_dtypes.bfloat16)

    pi = np.clip(pi0[:, :SLEN], 0.0, 1.0)  # [H, SLEN]
    pcm = np.empty((SLEN, 2 * H), np.float32)
    pcm[:, :H] = SCALE * pi.T
    pcm[:, H:] = SCALE * (1.0 - pi.T)

    s_idx = np.arange(128)[:, None]
    l_idx = np.arange(256)[None, :]
    mask0 = (s_idx <= l_idx).astype(np.float32)
    mask1 = (s_idx <= l_idx[:, :128]).astype(np.float32)
    ident = np.eye(128, dtype=np.float32)
    identb = np.eye(128, dtype=ml_dtypes.bfloat16)
    selmat = np.zeros((128, 24, 24), dtype=ml_dtypes.bfloat16)
    for j in range(24):
        selmat[:, j, j] = 1.0

    in_maps = []
    for c in range(NCORES):
        bsl = slice(c * BLOC, (c + 1) * BLOC)
        hc = h[:, bsl, :]
        hT = np.ascontiguousarray(hc.transpose(1, 2, 0)).astype(ml_dtypes.bfloat16)
        in_maps.append(
            dict(
                hT=hT,
                hN=np.ascontiguousarray(hc),
                w1=w1,
                w2=w2,
                pc=pcm,
                mask0=mask0,
                mask1=mask1,
                ident=ident,
                identb=identb,
                sel=selmat,
            )
        )
    return in_maps


def finalize_output(results, ln_gamma, ln_beta):
    outs = [results[c]["out"] for c in range(NCORES)]
    full = np.concatenate(outs, axis=1)  # [SLEN, BSZ, DM]
    g = np.asarray(ln_gamma, dtype=np.float32)
    bta = np.asarray(ln_beta, dtype=np.float32)
    if not (np.all(g == 1.0) and np.all(bta == 0.0)):
        full = full * g + bta
    return full.astype(np.float32)


def kernel(h, W_qkv, W_o, pi0, ln_gamma, ln_beta):
    nc = _get_program()
    in_maps = prepare_inputs(h, W_qkv, W_o, pi0, ln_gamma, ln_beta)
    res = run_bass_kernel_spmd(nc, in_maps, list(range(NCORES)))
    return finalize_output(res.results, ln_gamma, ln_beta)

